# revision 1
# baseline (speedup 1.0000x reference)
"""Bilateral cross-attention kernel for Trainium2 (8 NeuronCores).

Problem: x,y [2,256,64,64]; four attention branches (i,j) in {1,2}^2:
  f_i = wf_i @ src_i + bf_i   (src_1=x, src_2=y)     [32, 4096]
  g_j = wg_j @ src_j + bg_j                           [32, 4096]
  h_j = wh_j @ x + bh_j       (both h from x)         [32, 4096]
  attn_ij = softmax(f_i^T g_j, axis=-1)               [4096, 4096]
  o_ij = h_j @ attn_ij^T                              [32, 4096]
  contribution to out_i: wo_i[:, half_j] @ (wv_ij @ o_ij + bv_ij)
  x_out = x + contrib_11 + contrib_12 + bo1 ; y_out likewise.

Sharding: 8 cores = 2 batches x 4 branches; each core runs one full
attention with a fused output projection Wc = wo_half @ wv  [256, 32].
Host does only the final residual adds + bias vector adds.

On-core algorithm (flash-style, no max subtraction -- logits are bounded
by ~ +-25 so exp stays in fp32 range):
  S^T chunk [128 keys, 512 q] = g_chunk^T f_qblock  (K=32 row-tiled 3x)
  P = exp(S^T)  on ScalarE (ACT), PSUM->SBUF, one activate per 3 chunks
  acc [33, 512] += [h^T | 1]^T_chunk @ P_chunk   (ones row => softmax sum)
  out = (Wc @ acc[0:32]) * (1/acc[32]) broadcast
All attention matmuls use float32r (1 cycle/row, ~1e-4 relative error).
"""

import os
import numpy as np

BS, C, H, W = 2, 256, 64, 64
N = H * W            # 4096
CH = 32              # qkv channels
QB = 512             # query block (one PSUM bank of fp32)
NQB = N // QB        # 8
KC = 128             # key chunk (partition dim)
NKC = N // KC        # 32
GROUP = 3            # key chunks per round (3 PSUM banks, double buffered)

_CACHE = {}


def _groups():
    # first group takes the remainder (2 chunks), rest are GROUP-sized
    rem = NKC % GROUP or GROUP
    gs = [list(range(0, rem))]
    c = rem
    while c < NKC:
        gs.append(list(range(c, c + GROUP)))
        c += GROUP
    return gs


def build_nc():
    import concourse.bacc as bacc
    import concourse.mybir as mybir
    import concourse.tile as tile

    F32 = mybir.dt.float32
    F32R = mybir.dt.float32r
    EXP = mybir.ActivationFunctionType.Exp

    nc = bacc.Bacc("TRN2", target_bir_lowering=False)

    srcA = nc.dram_tensor("srcA", [C, N], F32R, kind="ExternalInput")
    srcB = nc.dram_tensor("srcB", [C, N], F32R, kind="ExternalInput")
    wq = nc.dram_tensor("wq", [2 * C, 3 * CH], F32R, kind="ExternalInput")
    wk = nc.dram_tensor("wk", [2 * C, 3 * CH], F32R, kind="ExternalInput")
    wv = nc.dram_tensor("wv", [C, CH + 2], F32R, kind="ExternalInput")
    wc = nc.dram_tensor("wc", [CH, C], F32R, kind="ExternalInput")
    bqk = nc.dram_tensor("bqk", [3 * CH, 2], F32, kind="ExternalInput")
    bh = nc.dram_tensor("bh", [1, CH + 1], F32, kind="ExternalInput")
    out = nc.dram_tensor("out", [C, N], F32, kind="ExternalOutput")

    groups = _groups()

    with tile.TileContext(nc) as tc:
        with (
            tc.tile_pool(name="src", bufs=1) as src,
            tc.tile_pool(name="persist", bufs=1) as persist,
            tc.tile_pool(name="ppool", bufs=5) as ppool,
            tc.tile_pool(name="opool", bufs=6) as opool,
            tc.tile_pool(name="tail", bufs=2) as tailp,
            tc.tile_pool(name="spool", bufs=2, space="PSUM") as spool,
            tc.tile_pool(name="accp", bufs=1, space="PSUM") as accp,
            tc.tile_pool(name="smallp", bufs=1, space="PSUM") as smallp,
        ):
            # ---------------- load inputs ----------------
            # Sources split into column blocks, emission interleaved so the
            # first blocks of all three tensors land early.
            NBLK = 8
            BW = N // NBLK
            # small weight/bias DMAs first so projections can start at once
            wq_sb = persist.tile([128, 4, 3 * CH], F32R)
            wk_sb = persist.tile([128, 4, 3 * CH], F32R)
            wv_sb = persist.tile([128, 2, CH + 2], F32R)
            nc.sync.dma_start(
                out=wq_sb, in_=wq[:, :].rearrange("(c p) w -> p c w", p=128)
            )
            nc.scalar.dma_start(
                out=wk_sb, in_=wk[:, :].rearrange("(c p) w -> p c w", p=128)
            )
            nc.sync.dma_start(
                out=wv_sb, in_=wv[:, :].rearrange("(c p) w -> p c w", p=128)
            )
            wc_sb = persist.tile([CH, C], F32R)
            nc.scalar.dma_start(out=wc_sb, in_=wc[:, :])
            bqk_sb = persist.tile([3 * CH, 2], F32)
            nc.sync.dma_start(out=bqk_sb, in_=bqk[:, :])
            bq_sb = bqk_sb[:, 0:1]
            bk_sb = bqk_sb[:, 1:2]
            # h bias row (+1.0 flag for the ones column) broadcast down partitions
            bh_sb = persist.tile([128, CH + 1], F32)
            nc.gpsimd.dma_start(out=bh_sb, in_=bh[:, :].to_broadcast((128, CH + 1)))
            ones32f = persist.tile([1, 32], F32)
            nc.vector.memset(ones32f, 1.0)
            ones32 = persist.tile([1, 32], F32R)
            nc.vector.tensor_copy(out=ones32, in_=ones32f)
            a_sb = src.tile([128, 2, N], F32R)   # [part, ch_chunk, n] = x[b]
            b_sb = src.tile([128, 2, N], F32R)   # y[b]
            dma_order = []
            for blk in range(NBLK):
                dma_order += [(a_sb, srcA, blk), (b_sb, srcB, blk)]
            for di, (t_sb, t_dr, blk) in enumerate(dma_order):
                bs_ = slice(BW * blk, BW * blk + BW)
                for cc in range(2):
                    nc.sync.dma_start(
                        out=t_sb[:, cc, bs_],
                        in_=t_dr[128 * cc : 128 * cc + 128, bs_],
                    )

            # ---------------- projection emitters ----------------
            # Weights arrive stacked 3x on the host so one matmul writes all
            # three partition blocks (row-tiled S^T matmuls need lhsT/rhs
            # replicated at partition blocks 0..2). Emission is interleaved
            # into the first q-block's rounds so attention starts early.
            fq3 = persist.tile([128, N], F32R)
            gk3 = persist.tile([128, N], F32R)
            haug = persist.tile([128, 33 * NKC], F32R)

            def emit_fproj(n, use_small=False):
                qs = slice(QB * n, QB * n + QB)
                if use_small:
                    ps_f = smallp.tile([128, 512], F32, tag="sm", name="ps_f")
                else:
                    ps_f = spool.tile([128, 1536], F32, tag="s", name="ps_f")
                for cc in range(4):
                    nc.tensor.matmul(
                        out=ps_f[0 : 3 * CH, 0:QB],
                        lhsT=wq_sb[:, cc, :],
                        rhs=(a_sb if cc < 2 else b_sb)[:, cc % 2, qs],
                        start=(cc == 0),
                        stop=(cc == 3),
                    )
                nc.vector.tensor_scalar_add(
                    out=fq3[0 : 3 * CH, qs], in0=ps_f[0 : 3 * CH, 0:QB], scalar1=bq_sb
                )

            def emit_gproj(n):
                ks = slice(QB * n, QB * n + QB)
                ps_g = spool.tile([128, 1536], F32, tag="s", name="ps_g")
                for cc in range(4):
                    nc.tensor.matmul(
                        out=ps_g[0 : 3 * CH, 0:QB],
                        lhsT=wk_sb[:, cc, :],
                        rhs=(a_sb if cc < 2 else b_sb)[:, cc % 2, ks],
                        start=(cc == 0),
                        stop=(cc == 3),
                    )
                nc.vector.tensor_scalar_add(
                    out=gk3[0 : 3 * CH, ks], in0=ps_g[0 : 3 * CH, 0:QB], scalar1=bk_sb
                )

            def emit_hproj(c):
                # h^T chunk + ones column at haug cols [33c, 33c+33)
                # (wv padded with zero cols; bias row carries the 1.0)
                ks = slice(KC * c, KC * c + KC)
                ps_h = smallp.tile([128, 512], F32, tag="sm", name="ps_h")
                for cc in range(2):
                    nc.tensor.matmul(
                        out=ps_h[:, 0 : CH + 2],
                        lhsT=a_sb[:, cc, ks],
                        rhs=wv_sb[:, cc, :],
                        start=(cc == 0),
                        stop=(cc == 1),
                    )
                nc.vector.tensor_add(
                    out=haug[:, 33 * c : 33 * c + 33],
                    in0=ps_h[:, 0 : CH + 1],
                    in1=bh_sb,
                )

            # ---------------- attention (software-pipelined emission) ----
            # Per round: S^T matmuls, exp, then the PV of the PREVIOUS round
            # (so PE never stalls on the current round's exp). The per-qblock
            # tail (normalize + Wc projection + store) is spread across the
            # next q-block's rounds. g/h projections interleave into qb 0.
            repeat = int(os.environ.get("BILATTN_TIMING_REPEAT", "1"))
            rounds = []
            for rep in range(repeat):
                for n in range(NQB):
                    for s, grp in enumerate(groups):
                        rounds.append((rep, n, s, grp))

            acc_of = {}     # n -> psum acc tile
            tail_of = {}    # n -> dict(state for the tail chain)
            pend_q = []
            next_g = 0

            def emit_pv(n, grp, p_sb):
                if grp[0] == 0:
                    acc_of[n] = accp.tile([33, QB], F32, tag="acc", name="acc")
                for i, c in enumerate(grp):
                    nc.tensor.matmul(
                        out=acc_of[n],
                        lhsT=haug[:, 33 * c : 33 * c + 33],
                        rhs=p_sb[:, 512 * i : 512 * i + 512],
                        start=(c == 0),
                        stop=(c == NKC - 1),
                    )

            def tail_a(n, last_qb=False):
                # reciprocal of the softmax sums (straight from PSUM, so it
                # starts as soon as the last PV lands), then evacuate acc.
                # For the final q-block the evacuation runs on the by-then
                # idle ScalarE so it overlaps the DVE reciprocal.
                with nc.allow_low_precision(reason="softmax recip rounds to f32r"):
                    recip = tailp.tile([1, QB], F32R, tag="recip", name="recip")
                    nc.vector.reciprocal(out=recip, in_=acc_of[n][32:33, :])
                acc_sb = tailp.tile([CH, QB], F32, tag="acc_sb", name="acc_sb")
                if last_qb:
                    nc.scalar.copy(out=acc_sb, in_=acc_of[n][0:CH, :])
                else:
                    nc.vector.tensor_copy(out=acc_sb, in_=acc_of[n][0:CH, :])
                tail_of[n] = {"acc_sb": acc_sb, "recip": recip}

            def tail_b(n):
                st = tail_of[n]
                rb = smallp.tile([128, 512], F32, tag="sm", name="rb")
                nc.tensor.matmul(
                    out=rb[0:32, :], lhsT=ones32, rhs=st["recip"],
                    start=True, stop=True,
                )
                attn_sb = tailp.tile([CH, QB], F32R, tag="attn", name="attn_sb")
                nc.vector.tensor_mul(
                    out=attn_sb, in0=st["acc_sb"], in1=rb[0:32, :]
                )
                st["attn_sb"] = attn_sb

            def tail_c(n, last_qb=False):
                st = tail_of.pop(n)
                qs = slice(QB * n, QB * n + QB)
                for m in range(2):
                    o_ps = smallp.tile([128, 512], F32, tag="sm", name="o_ps")
                    nc.tensor.matmul(
                        out=o_ps,
                        lhsT=wc_sb[:, 128 * m : 128 * m + 128],
                        rhs=st["attn_sb"],
                        start=True,
                        stop=True,
                    )
                    o_sb = opool.tile([128, QB], F32, tag="o", name="o_sb")
                    if last_qb and m == 0:
                        nc.scalar.copy(out=o_sb, in_=o_ps)
                    else:
                        nc.vector.tensor_copy(out=o_sb, in_=o_ps)
                    nc.sync.dma_start(out=out[128 * m : 128 * m + 128, qs], in_=o_sb)

            for c0 in range(3):
                emit_hproj(c0)
            next_h0 = 3
            emit_fproj(0)
            for rep, n, s, grp in rounds:
                first_sweep = rep == 0 and n == 0
                if rep == 0 and s == 1 and n + 1 < NQB:
                    emit_fproj(n + 1, use_small=(n > 0))
                if first_sweep:
                    # g chunks needed by this round's S^T; h chunks needed by
                    # next round's (lagged) PV
                    while next_g * QB < KC * (grp[-1] + 1):
                        emit_gproj(next_g)
                        next_g += 1
                    while next_h0 <= min(grp[-1] + GROUP, NKC - 1):
                        emit_hproj(next_h0)
                        next_h0 += 1
                qs = slice(QB * n, QB * n + QB)
                ncols = 512 * len(grp)
                s_ps = spool.tile([128, 1536], F32, tag="s", name="s_ps")
                for i, c in enumerate(grp):
                    nc.tensor.matmul(
                        out=s_ps[:, 512 * i : 512 * i + 512],
                        lhsT=gk3[32 * i : 32 * i + 32, KC * c : KC * c + KC],
                        rhs=fq3[32 * i : 32 * i + 32, qs],
                        start=True,
                        stop=True,
                        tile_position=(32 * i, 0),
                    )
                p_sb = ppool.tile([128, 1536], F32R, tag="p", name="p_sb")
                nc.scalar.activation(
                    out=p_sb[:, 0:ncols], in_=s_ps[:, 0:ncols], func=EXP
                )
                pend_q.append((n, grp, p_sb))
                if len(pend_q) > 2:
                    emit_pv(*pend_q.pop(0))
                # previous q-block's tail, spread across this one's rounds
                prev = n - 1 if n > 0 else (NQB - 1 if rep > 0 else None)
                if prev is not None and prev in (set(acc_of) | set(tail_of)):
                    if s == 1:
                        tail_a(prev)
                        acc_of.pop(prev, None)
                    elif s == 4:
                        tail_b(prev)
                    elif s == 6:
                        tail_c(prev)
            # flush
            for pv in pend_q:
                emit_pv(*pv)
            last = NQB - 1
            tail_a(last, last_qb=True)
            acc_of.pop(last, None)
            tail_b(last)
            tail_c(last, last_qb=True)

    nc.compile()
    return nc


def _get_nc():
    if "nc" not in _CACHE:
        _CACHE["nc"] = build_nc()
    return _CACHE["nc"]


def kernel(x, y, wf1, bf1, wg1, bg1, wh1, bh1, wf2, bf2, wg2, bg2, wh2, bh2,
           wv11, bv11, wv12, bv12, wv21, bv21, wv22, bv22, wo1, bo1, wo2, bo2):
    from concourse.bass_utils import run_bass_kernel_spmd

    f32 = np.float32
    x = np.asarray(x, f32)
    y = np.asarray(y, f32)
    wf = {1: np.asarray(wf1, f32), 2: np.asarray(wf2, f32)}
    bf = {1: np.asarray(bf1, f32), 2: np.asarray(bf2, f32)}
    wg = {1: np.asarray(wg1, f32), 2: np.asarray(wg2, f32)}
    bg = {1: np.asarray(bg1, f32), 2: np.asarray(bg2, f32)}
    wh = {1: np.asarray(wh1, f32), 2: np.asarray(wh2, f32)}
    bh_ = {1: np.asarray(bh1, f32), 2: np.asarray(bh2, f32)}
    wvv = {(1, 1): np.asarray(wv11, f32), (1, 2): np.asarray(wv12, f32),
           (2, 1): np.asarray(wv21, f32), (2, 2): np.asarray(wv22, f32)}
    bvv = {(1, 1): np.asarray(bv11, f32), (1, 2): np.asarray(bv12, f32),
           (2, 1): np.asarray(bv21, f32), (2, 2): np.asarray(bv22, f32)}
    wo = {1: np.asarray(wo1, f32), 2: np.asarray(wo2, f32)}
    bo = {1: np.asarray(bo1, f32), 2: np.asarray(bo2, f32)}

    src = {1: x, 2: y}
    branches = [(1, 1), (1, 2), (2, 1), (2, 2)]

    in_maps = []
    for b in range(BS):
        for (i, j) in branches:
            wc_np = wo[i][:, 256 * (j - 1) : 256 * j] @ wvv[(i, j)]  # [256, 32]
            wq_e = np.zeros((2 * C, 3 * CH), f32)
            wq_e[(i - 1) * C : i * C, :] = np.tile(wf[i].T, (1, 3))
            wk_e = np.zeros((2 * C, 3 * CH), f32)
            wk_e[(j - 1) * C : j * C, :] = np.tile(wg[j].T, (1, 3))
            m = {
                "srcA": np.ascontiguousarray(x[b].reshape(C, N)),
                "srcB": np.ascontiguousarray(y[b].reshape(C, N)),
                "wq": wq_e,
                "wk": wk_e,
                "wv": np.ascontiguousarray(
                    np.concatenate(
                        [wh[j].T, np.zeros((C, 2), f32)], axis=1
                    )
                ),
                "wc": np.ascontiguousarray(wc_np.T),
                "bqk": np.ascontiguousarray(
                    np.stack(
                        [np.tile(bf[i], 3), np.tile(bg[j], 3)], axis=1
                    ).astype(f32)
                ),
                "bh": np.ascontiguousarray(
                    np.concatenate([bh_[j], [1.0]]).astype(f32).reshape(1, CH + 1)
                ),
            }
            in_maps.append(m)

    nc = _get_nc()
    trace = os.environ.get("KERNEL_TRACE", "0")
    kwargs = {}
    if trace == "1":
        kwargs = dict(trace=True, trace_cores=[0])
    elif trace == "all":
        kwargs = dict(trace=True, trace_cores=list(range(8)))
    res = run_bass_kernel_spmd(nc, in_maps, core_ids=list(range(8)), **kwargs)
    _CACHE["last_result"] = res

    parts = {}
    k = 0
    for b in range(BS):
        for (i, j) in branches:
            parts[(b, i, j)] = res.results[k]["out"]
            k += 1

    outs = []
    for i, resid in ((1, x), (2, y)):
        biasvec = (
            wo[i][:, 0:256] @ bvv[(i, 1)]
            + wo[i][:, 256:512] @ bvv[(i, 2)]
            + bo[i]
        ).astype(f32)
        o = np.empty_like(resid)
        for b in range(BS):
            acc = parts[(b, i, 1)] + parts[(b, i, 2)] + biasvec[:, None]
            o[b] = resid[b] + acc.reshape(C, H, W)
        outs.append(o)
    return tuple(outs)

